# revision 10
# baseline (speedup 1.0000x reference)
"""Trainium2 Bass kernel: DiscreteEmbedding (rect-window embedding lookup).

Math (matches the jax reference):
    xs  = x * 2048;  y = xs + 0.5
    i_lo = ceil(y)-1, i_hi = floor(y)
    out[t] = 0.5*T[i_lo] + 0.5*T[i_hi]      (T extended with zero row 2048)
Non-boundary tokens (y non-integer): i_lo == i_hi -> out = T[i_lo].
Boundary tokens (y integer, ~1/8192 of tokens): out = avg of two rows.

Device strategy (8 cores, data-parallel over tokens):
  - Combined table TC built on the HOST (depends only on the weights):
      TC[0:2048] = T;  TC[2048] = 0;  TC[2049+k] = (T[k]+T[k+1])/2
    stored as bf16 bit patterns in uint16 (the gather is a pure byte
    mover; bf16 halves both gather-read and store HBM traffic).
    One gather per token at idx2 = i_lo + 2049*(y integer).
  - SWDGE dma_gather with prepare_only=True: desc-gen decoupled from the
    SDMA drain runs at ~2.7 ns/idx (vs ~8.8 ns/idx drain-paced), on all
    4 queues' Q7 core pairs concurrently. trigger_dma fires each wave;
    the drain of wave 1 overlaps the desc-gen of wave 2.
  - Tile does not gate SBUF reads on a prep's deferred DMA, so each
    chunk's store is explicitly gated on the per-queue DMA-completion
    semaphore (16 incs per chunk, ring-FIFO order within a queue).
  - x is passed wrapped [16,512] replicated to [128,512]: full-width DVE
    index math, and partitions 16..127 double as the per-Q7-core replicas
    of the int16 index buffer that dma_gather expects. The index math
    overlaps the fixed ~10us Q7 library load.
  - Stores alternate between the SP (sync) and ACT (scalar) HWDGE rings;
    host un-permutes rows (free) while un-sharding and widens bf16->f32.
"""

import numpy as np

import concourse.mybir as mybir
import concourse.tile as tile
from concourse.tile import add_dep_helper
from concourse import bacc, bass_utils

N_CORES = 8
B, S = 32, 2048
V, D = 2048, 128
TOK = B * S                 # 65536 tokens total
TPC = TOK // N_CORES        # 8192 tokens per core
SPC = TPC // 16             # 512: free dim of the wrapped [16, 512] x layout
ABASE = V + 1               # 2049: base row of the averaged-pair table
VEXT = 4224                 # TC rows (>= 2*V+1, multiple of 128)
NQ = 4                      # SWDGE queues
PREP = False                # prepare_only + trigger architecture

# (j_block_start, j_block_count, queue) per chunk, in dispatch order.
# Round-robin with the queue-0 chunk (synchronous desc-gen on the POOL
# NX) last in each round so queues 1-3 stay fed. 16 j-blocks per queue
# in 3 chunks (6,5,5) — the s2m desc-gen runs at a hard ~8 ns/desc per
# queue pair, so fewer chunks means less per-chunk fixed overhead.
# Loads per queue follow measured desc-gen rates (q1 fastest, q0/q2 slowest):
# q1: 17 jb, q2: 16, q3: 15, q0: 16.
WAVES = [
    [(0, 6, 1), (6, 6, 2), (12, 5, 3), (17, 6, 0),
     (23, 6, 1), (29, 5, 2), (34, 5, 3), (39, 5, 0),
     (44, 5, 1), (49, 5, 2), (54, 5, 3), (59, 5, 0)],
]
assert sum(c[1] for w in WAVES for c in w) == TPC // 128

F32 = mybir.dt.float32
I32 = mybir.dt.int32
I16 = mybir.dt.int16
U16 = mybir.dt.uint16
OP = mybir.AluOpType


def build():
    nc = bacc.Bacc(
        "TRN2",
        target_bir_lowering=False,
        debug=False,
        num_devices=N_CORES,
        num_swdge_queues=NQ,
    )
    xr = nc.dram_tensor("xr", [128, SPC], F32, kind="ExternalInput")
    tcb = nc.dram_tensor("tcb", [VEXT, D], U16, kind="ExternalInput")
    out = nc.dram_tensor("out", [TPC, D], U16, kind="ExternalOutput")

    with tile.TileContext(nc) as tc:
        with tc.tile_pool(name="sb", bufs=1) as sb, tc.tile_pool(name="g", bufs=1) as gp:
            # Warm-up: pay the Q7 library-load + per-queue ring init early,
            # overlapped with the x load and index math.
            zidx = sb.tile([128, 16], I16)
            nc.gpsimd.memset(zidx[:], 0)
            warm = {}
            for q in [1, 2, 3, 0]:
                wg = sb.tile([128, D], U16, tag=f"warm{q}")
                warm[q] = nc.gpsimd.dma_gather(
                    wg[:].rearrange("p (j d) -> p j d", d=D),
                    tcb[:],
                    zidx[:, 0:1],
                    num_idxs=16,
                    num_idxs_reg=16,
                    elem_size=D,
                    single_packet=False,
                    queue_num=q,
                )

            nidx_regs = {
                n: nc.gpsimd.to_reg(128 * n)
                for n in sorted({c[1] for w in WAVES for c in w})
            }

            xt = sb.tile([128, SPC], F32)
            nc.sync.dma_start(out=xt[:], in_=xr[:])

            # ---- index math (fp32, exact): y = x*2048 + 0.5; i0 = rne(y);
            # fix up to i_lo = ceil(y)-1; idx2 = i_lo + 2049*(y integer). ----
            y = sb.tile([128, SPC], F32)
            nc.vector.tensor_scalar(y[:], xt[:], 2048.0, 0.5, op0=OP.mult, op1=OP.add)
            i0 = sb.tile([128, SPC], I32)
            nc.vector.tensor_copy(i0[:], y[:])
            f0 = sb.tile([128, SPC], F32)
            nc.vector.tensor_copy(f0[:], i0[:])
            lt = sb.tile([128, SPC], F32)
            nc.vector.tensor_tensor(lt[:], f0[:], y[:], op=OP.is_lt)
            bnd = sb.tile([128, SPC], F32)
            nc.vector.tensor_tensor(bnd[:], f0[:], y[:], op=OP.is_equal)
            lf = sb.tile([128, SPC], F32)
            nc.vector.scalar_tensor_tensor(
                out=lf[:], in0=f0[:], scalar=-1.0, in1=lt[:], op0=OP.add, op1=OP.add
            )
            idx16 = sb.tile([128, SPC], I16)
            nc.vector.scalar_tensor_tensor(
                out=idx16[:], in0=bnd[:], scalar=float(ABASE), in1=lf[:],
                op0=OP.mult, op1=OP.add,
            )

            # ---- prep / trigger / store ----
            out_v = out[:].rearrange("(p j) d -> p (j d)", p=128)
            qsem = {q: nc.alloc_semaphore(f"gsem{q}") for q in range(NQ)}
            qcnt = {q: 0 for q in range(NQ)}
            ci = 0
            stores = []  # (chunk_idx, j0, jbc, queue, wait_val, g_tile, first_of_q)
            for wave in WAVES:
                preps = []
                for (j0, jbc, q) in wave:
                    g = gp.tile([128, jbc * D], U16, tag=f"g{ci}")
                    if PREP:
                        gi = nc.gpsimd.dma_gather(
                            g[:].rearrange("p (j d) -> p j d", d=D),
                            tcb[0 : ABASE + V],
                            idx16[:, j0 * 8 : (j0 + jbc) * 8],
                            num_idxs=128 * jbc,
                            num_idxs_reg=nidx_regs[jbc],
                            elem_size=D,
                            single_packet=False,
                            queue_num=q,
                            prepare_only=True,
                            sem=qsem[q],
                        )
                    else:
                        gi = nc.gpsimd.dma_gather(
                            g[:].rearrange("p (j d) -> p j d", d=D),
                            tcb[0 : ABASE + V],
                            idx16[:, j0 * 8 : (j0 + jbc) * 8],
                            num_idxs=128 * jbc,
                            num_idxs_reg=nidx_regs[jbc],
                            elem_size=D,
                            single_packet=False,
                            queue_num=q,
                        )
                    if PREP:
                        # never run prep desc-gen before the library load done
                        add_dep_helper(gi.ins, warm[q].ins, True, "lib/ring ready")
                    preps.append(gi)
                    qcnt[q] += 1
                    stores.append((ci, j0, jbc, q, 16 * qcnt[q], g, gi))
                    ci += 1
                if PREP:
                    for q in sorted({c[2] for c in wave}):
                        nc.gpsimd.trigger_dma(count=None, queue_num=q)

            for (k, j0, jbc, q, wv, g, gi) in stores:
                eng = nc.sync if k % 2 == 0 else nc.scalar
                if PREP:
                    w = eng.wait_ge(qsem[q], wv)
                    add_dep_helper(w.ins, gi.ins, True, "after prep")
                    st = eng.dma_start(
                        out=out_v[:, j0 * D : (j0 + jbc) * D], in_=g[:]
                    )
                    add_dep_helper(st.ins, w.ins, True, "gated on drain")
                else:
                    eng.dma_start(
                        out=out_v[:, j0 * D : (j0 + jbc) * D], in_=g[:]
                    )
    nc.compile()
    return nc


_NC = None


def _row_perm():
    """out row r holds gather position i(r); position i handles token
    t(i) = (i%16)*512 + i//16 (x wrapped [16,512] across partitions)."""
    r = np.arange(TPC)
    p, j = r // 64, r % 64
    i = j * 128 + p
    return (i % 16) * SPC + i // 16  # token index held at row r


def _f32_to_bf16_bits(a):
    bits = np.ascontiguousarray(a, dtype=np.float32).view(np.uint32)
    return (((bits + 0x7FFF + ((bits >> 16) & 1)) >> 16) & 0xFFFF).astype(np.uint16)


def _build_tc(t):
    tc = np.zeros((VEXT, D), dtype=np.float32)
    tc[0:V] = t
    ext = np.vstack([t, np.zeros((1, D), dtype=np.float32)])
    tc[ABASE : ABASE + V] = 0.5 * (ext[0:V] + ext[1 : V + 1])
    return _f32_to_bf16_bits(tc)


def kernel(x, time_embedding):
    global _NC
    x = np.ascontiguousarray(np.asarray(x, dtype=np.float32))
    t = np.ascontiguousarray(np.asarray(time_embedding, dtype=np.float32))
    tcb = _build_tc(t)
    xf = x.reshape(-1)
    in_maps = []
    for c in range(N_CORES):
        xc = xf[c * TPC : (c + 1) * TPC].reshape(16, SPC)
        in_maps.append({"xr": np.ascontiguousarray(np.tile(xc, (8, 1))), "tcb": tcb})

    if _NC is None:
        _NC = build()
    res = bass_utils.run_bass_kernel_spmd(_NC, in_maps, core_ids=list(range(N_CORES)))
    global _LAST_RES
    _LAST_RES = res

    tkn = _row_perm()
    outs = []
    for c in range(N_CORES):
        oc = np.asarray(res.results[c]["out"])  # [TPC, D] uint16 (bf16 bits)
        of = (oc.astype(np.uint32) << 16).view(np.float32)
        full = np.empty_like(of)
        full[tkn] = of
        outs.append(full)
    return np.concatenate(outs, axis=0).reshape(B, S, D)
